# revision 1
# baseline (speedup 1.0000x reference)
"""NT-Xent / SimCLR contrastive loss on 8 Trainium2 NeuronCores.

Strategy (data-parallel over rows of the concatenated representations):
  - Host: reps = concat(z_i, z_j) -> [8192, 512] fp32. Core i receives
    reps rolled by -1024*i rows so that *its* 1024 rows sit at rows 0..1023.
    This makes the SPMD program identical on every core (static offsets),
    and the rolled positive pairs land at a fixed diagonal (col = row+4096).
  - Device (per core):
      phase A: load rows with 2D cast DMAs (fp32 DRAM -> bf16 SBUF),
               compute 1/||row|| via fused square+rowsum (DVE
               scalar_tensor_tensor) and exp(-0.5*ln(n2)) (ACT, one table
               set), scale rows (split DVE/ACT), then transpose to
               repsT [512, 8192] with SBUF->SBUF xbar DMA transposes of
               [128,128] blocks (no DRAM bounce).
      phase B: sim slice = repsT[:, 0:1024].T @ repsT via bf16 matmuls
               accumulated over the 4 K-chunks into [128, 2048] PSUM tiles;
               ACT computes exp(2*sim) with fused row-sum (accum_out);
               DVE extracts the self/positive diagonals with an identity
               mask + fused reduce.
      epilogue: denom = rowsum - exp(2*sim_self); partial row loss is
               ln(denom) - 2*pos; partition-sum via a ones-matmul; DMA the
               per-core scalar out.
  - Host: loss = sum(core partials) / 8192.
"""

import sys
import threading
from unittest import mock

sys.path.insert(0, "/opt/trn_rl_repo")

import numpy as np  # noqa: E402

import concourse.tile as tile  # noqa: E402
from concourse import bacc, mybir  # noqa: E402
from concourse.bass_utils import run_bass_kernel_spmd  # noqa: E402
from concourse.hw_specs import get_activation_tables  # noqa: E402
from concourse.masks import make_identity  # noqa: E402
from contextlib import ExitStack  # noqa: E402

P = 128
D = 512
TWO_N = 8192
N_CORES = 8
ROWS_PER_CORE = TWO_N // N_CORES  # 1024
T_INV = 2.0  # 1 / temperature (0.5)

KC = D // P  # 4 contraction chunks of 128
NB = 4  # big column blocks (= row groups in phase A)
CB = TWO_N // NB  # 2048 columns per block / rows per group
TPG = CB // P  # 16 [128, 512] row tiles per group
MB = ROWS_PER_CORE // P  # 8 m-blocks of 128 rows per core
NS = CB // 512  # 4 matmul sub-columns of 512 per block

FP32 = mybir.dt.float32
BF16 = mybir.dt.bfloat16
AF = mybir.ActivationFunctionType
ALU = mybir.AluOpType
AX = mybir.AxisListType


def _filtered_activation_tables(arch):
    """Steer every Exp/Ln/Copy activation to the one table set containing
    both Exp and Ln, so the table-load pass cannot thrash between sets.
    The dict must keep ALL sets in original order: the set id emitted into
    the NEFF is the index into act_info.json's list."""
    tables = get_activation_tables(arch)
    target = None
    for name, funcs in tables.items():
        if AF.Exp in funcs and AF.Ln in funcs:
            target = name
            break
    if target is None:
        return tables
    steer = {AF.Exp, AF.Ln, AF.Copy, AF.Identity}
    return {
        name: (funcs if name == target else funcs - steer)
        for name, funcs in tables.items()
    }


def _build_kernel():
    nc = bacc.Bacc("TRN2", target_bir_lowering=False, debug=False,
                   num_devices=N_CORES)
    reps = nc.dram_tensor("reps", [TWO_N, D], FP32, kind="ExternalInput").ap()
    out = nc.dram_tensor("out", [1, 1], FP32, kind="ExternalOutput").ap()

    with tile.TileContext(nc) as tc, ExitStack() as ctx:
        rows_pool = ctx.enter_context(tc.tile_pool(name="rows", bufs=2))
        normed_pool = ctx.enter_context(tc.tile_pool(name="normed", bufs=2))
        sq_pool = ctx.enter_context(tc.tile_pool(name="sq", bufs=2))
        stats_pool = ctx.enter_context(tc.tile_pool(name="stats", bufs=1))
        repsT_pool = ctx.enter_context(tc.tile_pool(name="repsT", bufs=1))
        dram_pool = ctx.enter_context(
            tc.tile_pool(name="scratch", bufs=KC * NB, space="DRAM"))
        psum_pool = ctx.enter_context(
            tc.tile_pool(name="psum", bufs=2, space="PSUM"))
        exp_pool = ctx.enter_context(tc.tile_pool(name="exp", bufs=2))
        junk_pool = ctx.enter_context(tc.tile_pool(name="junk", bufs=2))
        epi_pool = ctx.enter_context(tc.tile_pool(name="epi", bufs=1))

        # --- constants -----------------------------------------------------
        ident = stats_pool.tile([P, P], FP32, tag="ident", name="ident")
        make_identity(nc, ident[:])
        ones = stats_pool.tile([P, 1], FP32, tag="ones", name="ones")
        nc.gpsimd.memset(ones[:], 1.0)

        # accumulators for the main loop
        rs_all = stats_pool.tile([P, MB * NB], FP32, tag="rs", name="rs_all")
        e_self = stats_pool.tile([P, MB], FP32, tag="eself", name="e_self")
        pos = stats_pool.tile([P, MB], FP32, tag="pos", name="pos")

        # repsT[k][g]: [128, 2048] bf16 — chunk k (rows k*128..k*128+127 of
        # the transposed matrix) for columns g*2048..(g+1)*2048.
        repsT = [[repsT_pool.tile([P, CB], BF16, tag=f"rT{k}_{g}",
                                  name=f"repsT_{k}_{g}")
                  for g in range(NB)]
                 for k in range(KC)]
        # repsT0[k]: [128, 1024] bf16 — unpermuted (row-major) copy of the
        # core's own 1024 columns, so matmul lhsT slices are contiguous
        # (strided LDWEIGHTS measured ~45% slower).
        repsT0 = [repsT_pool.tile([P, ROWS_PER_CORE], BF16, tag=f"rTz_{k}",
                                  name=f"repsT0_{k}")
                  for k in range(KC)]

        # --- phase A: normalize rows, transpose via SBUF xbar --------------
        for g in range(NB):
            rows_g = rows_pool.tile([P, TPG * D], FP32, tag="rows",
                                    name=f"rows_{g}")
            src = reps[g * CB:(g + 1) * CB, :].rearrange(
                "(t p) d -> p t d", p=P)
            nc.sync.dma_start(
                out=rows_g[:].rearrange("p (t d) -> p t d", d=D), in_=src)

            n2 = stats_pool.tile([P, TPG], FP32, tag="n2", bufs=2,
                                 name=f"n2_{g}")
            for t in range(TPG):
                sq = sq_pool.tile([P, D], BF16, tag="sq", name=f"sq_{g}_{t}")
                rt = rows_g[:, t * D:(t + 1) * D]
                nc.vector.scalar_tensor_tensor(
                    out=sq[:], in0=rt, scalar=1.0, in1=rt,
                    op0=ALU.mult, op1=ALU.mult, accum_out=n2[:, t:t + 1])
            # inv = n2 ** -0.5 = exp(-0.5 * ln(n2)); Ln+Exp share one ACT
            # table set (forced via _filtered_activation_tables).
            lnn = stats_pool.tile([P, TPG], FP32, tag="lnn", bufs=2,
                                  name=f"lnn_{g}")
            nc.scalar.activation(lnn[:], n2[:], AF.Ln)
            inv = stats_pool.tile([P, TPG], FP32, tag="inv", bufs=2,
                                  name=f"inv_{g}")
            nc.scalar.activation(inv[:], lnn[:], AF.Exp, scale=-0.5)

            normed_g = normed_pool.tile([P, TPG * D], BF16, tag="normed",
                                        name=f"normed_{g}")
            for t in range(TPG):
                src_t = rows_g[:, t * D:(t + 1) * D]
                dst_t = normed_g[:, t * D:(t + 1) * D]
                if t % 2 == 0:
                    nc.vector.tensor_scalar_mul(dst_t, src_t, inv[:, t:t + 1])
                else:
                    # ACT path: Copy is present in every table set.
                    nc.scalar.activation(dst_t, src_t, AF.Copy,
                                         scale=inv[:, t:t + 1])
            # Bounce through DRAM per d-chunk, permuted so both the store
            # (4 KiB runs per partition) and the transpose read (fully
            # contiguous) are DMA-friendly. Scratch row q = p*16 + t holds
            # normalized row t*128 + p, so repsT group column q <-> global
            # row (q%16)*128 + q//16.
            nview = normed_g[:].rearrange("p (t e) -> p t e", e=D)
            for k in range(KC):
                scr = dram_pool.tile([CB, P], BF16, tag=f"scr{k}_{g}",
                                     name=f"scr_{k}_{g}")
                nc.sync.dma_start(
                    out=scr[:].rearrange("(p t) c -> p t c", p=P),
                    in_=nview[:, :, k * P:(k + 1) * P])
                nc.sync.dma_start_transpose(repsT[k][g][:], scr[:])
            if g == 0:
                # un-permute the core's own 1024 columns for contiguous
                # lhsT: repsT0 col (m*128+j) = repsT[.][0] col (16j+m)
                for k in range(KC):
                    nc.vector.tensor_copy(
                        repsT0[k][:].rearrange("p (m j) -> p m j", j=P),
                        repsT[k][0][:].rearrange(
                            "p (j m) -> p m j", m=TPG)[:, :MB, :])

        # --- phase B: similarity matmuls + softmax statistics --------------
        # Scratch-permuted column q of a repsT group holds global row
        # (q%16)*128 + q//16, so the 128 columns for m-block rows
        # m*128..m*128+127 sit at positions 16*j + m (j = psum row).
        def colsel(ap_2d, m):
            return ap_2d.rearrange("p (j s) -> p s j", s=TPG)[:, m, :]

        for nb in range(NB):
            for m in range(MB):
                ps = psum_pool.tile([P, CB], FP32, tag="ps",
                                    name=f"ps_{nb}_{m}")
                for ns in range(NS):
                    for k in range(KC):
                        nc.tensor.matmul(
                            ps[:, ns * 512:(ns + 1) * 512],
                            lhsT=repsT0[k][:, m * P:(m + 1) * P],
                            rhs=repsT[k][nb][:, ns * 512:(ns + 1) * 512],
                            start=(k == 0), stop=(k == KC - 1))
                et = exp_pool.tile([P, CB], BF16, tag="et", name=f"et_{nb}_{m}")
                nc.scalar.activation(
                    et[:], ps[:], AF.Exp, scale=T_INV,
                    accum_out=rs_all[:, m * NB + nb:m * NB + nb + 1])
                if nb == 0:
                    # self-similarity column: global col = row = m*128 + j,
                    # at permuted position 16*j + m.
                    junk = junk_pool.tile([P, P], FP32, tag="junk",
                                          name=f"junk_s_{m}")
                    nc.vector.scalar_tensor_tensor(
                        out=junk[:], in0=colsel(et[:], m),
                        scalar=1.0, in1=ident[:],
                        op0=ALU.mult, op1=ALU.mult,
                        accum_out=e_self[:, m:m + 1])
                if nb == 2:
                    # positive column: global col = 4096 + row, in-group
                    # offset = row -> same permuted position 16*j + m.
                    junk = junk_pool.tile([P, P], FP32, tag="junk",
                                          name=f"junk_p_{m}")
                    nc.vector.scalar_tensor_tensor(
                        out=junk[:], in0=colsel(ps[:], m),
                        scalar=1.0, in1=ident[:],
                        op0=ALU.mult, op1=ALU.mult,
                        accum_out=pos[:, m:m + 1])

        # --- epilogue ------------------------------------------------------
        sums = epi_pool.tile([P, MB], FP32, tag="sums", name="sums")
        nc.vector.tensor_reduce(
            sums[:], rs_all[:].rearrange("p (m b) -> p m b", b=NB),
            axis=AX.X, op=ALU.add)
        denom = epi_pool.tile([P, MB], FP32, tag="denom", name="denom")
        nc.vector.tensor_sub(denom[:], sums[:], e_self[:])
        ld = epi_pool.tile([P, MB], FP32, tag="ld", name="ld")
        nc.scalar.activation(ld[:], denom[:], AF.Ln)
        # partial = ld - 2*pos = (pos * -2) + ld
        part = epi_pool.tile([P, MB], FP32, tag="part", name="part")
        nc.vector.scalar_tensor_tensor(
            out=part[:], in0=pos[:], scalar=-T_INV, in1=ld[:],
            op0=ALU.mult, op1=ALU.add)
        rowtot = epi_pool.tile([P, 1], FP32, tag="rowtot", name="rowtot")
        nc.vector.tensor_reduce(rowtot[:], part[:], axis=AX.X, op=ALU.add)
        pfin = psum_pool.tile([P, CB], FP32, tag="ps", name="pfin")
        nc.tensor.matmul(pfin[:1, :1], lhsT=ones[:], rhs=rowtot[:])
        out_sb = epi_pool.tile([1, 1], FP32, tag="osb", name="out_sb")
        nc.vector.tensor_copy(out_sb[:], pfin[:1, :1])
        nc.sync.dma_start(out=out[:, :], in_=out_sb[:])

    with mock.patch("concourse.bacc.get_activation_tables",
                    _filtered_activation_tables):
        nc.compile()
    return nc


_CACHE_LOCK = threading.Lock()
_CACHED_NC = None


def _get_nc():
    global _CACHED_NC
    with _CACHE_LOCK:
        if _CACHED_NC is None:
            _CACHED_NC = _build_kernel()
        return _CACHED_NC


def _run(inputs, trace=False):
    z_i = np.asarray(inputs["z_i"], dtype=np.float32)
    z_j = np.asarray(inputs["z_j"], dtype=np.float32)
    reps = np.concatenate([z_i, z_j], axis=0)
    in_maps = [
        {"reps": np.ascontiguousarray(
            np.roll(reps, -ROWS_PER_CORE * i, axis=0))}
        for i in range(N_CORES)
    ]
    nc = _get_nc()
    res = run_bass_kernel_spmd(nc, in_maps, list(range(N_CORES)), trace=trace)
    partials = [float(res.results[i]["out"][0, 0]) for i in range(N_CORES)]
    loss = np.float32(np.sum(np.asarray(partials, dtype=np.float64)) / TWO_N)
    return loss, res


def kernel(**inputs):
    loss, _ = _run(inputs, trace=False)
    return np.asarray(loss, dtype=np.float32)

